# revision 28
# baseline (speedup 1.0000x reference)
"""Causal multi-head attention (B=32,T=512,C=1024,H=16,D=64) on 8 TRN2 cores.

Strategy: pure data-parallel over the batch axis (4 batches per core, no
collectives). Per core, per batch:
  - Q^T/K^T projections run in fp8(e4m3) DoubleRow mode: contraction 256 per
    pass (2x fewer PE matmuls). fp8 noise only perturbs attention logits
    (sigma~0.25) so end-to-end rel-err stays ~1.3e-2 (gate 2e-2). Weights are
    pre-scaled x32 on the host; the exp() activation scale divides it back out.
  - V and the output projection stay bf16 (their noise hits the output
    directly).
  - K^T is stored pair-packed: head 2m on partitions 0:64, head 2m+1 on
    64:128. scores^T then runs as K=64 row-tiled matmul PAIRS
    (tile_position (0,0)/(64,0)) -- two heads stream concurrently through
    disjoint row-groups of the PE array, ~2x scores throughput, and no
    zero-padding matmul waste.
  - scores^T packs the 4 causal diagonal blocks contiguously at [0,512) so
    the 0/1 triangular mask is ONE gpsimd multiply per head (not 4).
  - softmax without max-subtraction (logits bounded); attn@[V|1] accumulates
    all 4 t-chunks of a head into a single PSUM bank, so normalization is one
    batched reciprocal + one broadcast tensor_mul per head (stride-0 AP).
  - head-concat transpose via one batched DMA-transpose per t-chunk; final
    projection with bias folded in via a K=128 matmul; fp32 output.
"""

import sys

if "/opt/trn_rl_repo" not in sys.path:
    sys.path.insert(0, "/opt/trn_rl_repo")

import numpy as np
import ml_dtypes

B, T, C = 32, 512, 1024
H, D = 16, 64
HD = H * D
NCORES = 8
B_LOC = B // NCORES
SW = 32.0  # host-side prescale of wq/wk before fp8 cast

_CACHE = {}


def build_nc(b_loc=B_LOC):
    import concourse.mybir as mybir
    from concourse import bacc
    from concourse.bass import ds, ts
    from concourse.tile import TileContext

    f32 = mybir.dt.float32
    bf16 = mybir.dt.bfloat16
    f8 = mybir.dt.float8e4
    AF = mybir.ActivationFunctionType
    DR = mybir.MatmulPerfMode.DoubleRow

    KO = C // 128  # 8 contraction chunks
    KO2 = KO // 2  # 4 DoubleRow chunks (K=256 each)
    MO = HD // 128  # 8 output-row chunks
    TCH = T // 128  # 4 t-chunks
    EXP_SCALE = 1.0 / (float(np.sqrt(C)) * SW * SW)

    # scores^T causal packing, diagonal-blocks-first:
    #   cols [128j, 128j+128)    : diagonal block of s-chunk j  (j=0..3)
    #   cols [OD[j], OD[j]+ODW[j]): off-diagonal strip of s-chunk j covering
    #                              t in [128(j+1), T)           (j=0..2)
    # Bank layout (2KB fp32 = 512 cols): [0,512) bank0; [512,896)+[896,1024)
    # bank1; [1024,1280) bank2 -- no matmul output crosses a bank boundary.
    OD = [512, 1024, 896]
    ODW = [384, 256, 128]
    PACK = 1280

    def av_block(i, j):
        # column offset of the aT block for (t-chunk i, s-chunk j), j<=i
        return 128 * i if i == j else OD[j] + 128 * (i - j - 1)

    nc = bacc.Bacc("TRN2", target_bir_lowering=False)
    xT = nc.dram_tensor("xT", [b_loc, C, T], bf16, kind="ExternalInput")
    xT8 = nc.dram_tensor("xT8", [b_loc, C, T], f8, kind="ExternalInput")
    wq8 = nc.dram_tensor("wq8", [C, HD], f8, kind="ExternalInput")
    wk8 = nc.dram_tensor("wk8", [C, HD], f8, kind="ExternalInput")
    wv = nc.dram_tensor("wv", [C, HD], bf16, kind="ExternalInput")
    wp = nc.dram_tensor("wp", [C, C], bf16, kind="ExternalInput")
    bp = nc.dram_tensor("bp", [1, C], bf16, kind="ExternalInput")
    mask4 = nc.dram_tensor("mask4", [128, 512], bf16, kind="ExternalInput")
    out = nc.dram_tensor("out", [b_loc, T, C], bf16, kind="ExternalOutput")

    with TileContext(nc) as tc:
        with (
            tc.tile_pool(name="weights", bufs=1) as wpool,
            tc.tile_pool(name="acts", bufs=2) as xpool,
            tc.tile_pool(name="attn", bufs=4) as apool,
            tc.tile_pool(name="small", bufs=8) as spool,
            tc.tile_pool(name="outs", bufs=2) as opool,
            tc.tile_pool(name="psS", bufs=2, space="PSUM") as psA,
            tc.tile_pool(name="ps1", bufs=2, space="PSUM") as psB,
        ):
            # ---- persistent weights ----
            # DMA order matters for the pipeline head: tiny tensors first,
            # then batch 0's fp8 x^T interleaved with wq8 so the first Q
            # matmul chain starts as soon as its operands land.
            wq8_sb = wpool.tile([128, KO, HD], f8, name="wq8_sb")
            wk8_sb = wpool.tile([128, KO, HD], f8, name="wk8_sb")
            wv_sb = wpool.tile([128, KO, HD], bf16, name="wv_sb")
            wp_sb = wpool.tile([128, KO, C], bf16, name="wp_sb")
            xT80_sb = xpool.tile([128, KO, T], f8, name="xT80_sb", tag="xT8")
            xT0_sb = xpool.tile([128, KO, T], bf16, name="xT0_sb", tag="xT")
            bp1_sb = wpool.tile([1, C], bf16, name="bp1_sb")
            nc.sync.dma_start(out=bp1_sb, in_=bp[:])
            mask4_sb = wpool.tile([128, 512], bf16, name="mask4_sb")
            nc.sync.dma_start(out=mask4_sb, in_=mask4[:])
            # single-instruction DMAs: each dma_start trigger costs ~0.5us on
            # the sync queue, so chunked loads delay the first matmul by ~5us
            nc.sync.dma_start(
                out=xT80_sb, in_=xT8[0].rearrange("(ko p) t -> p ko t", p=128)
            )
            nc.sync.dma_start(
                out=wq8_sb, in_=wq8[:].rearrange("(ko p) n -> p ko n", p=128)
            )
            nc.sync.dma_start(
                out=wk8_sb, in_=wk8[:].rearrange("(ko p) n -> p ko n", p=128)
            )
            nc.sync.dma_start(
                out=xT0_sb, in_=xT[0].rearrange("(ko p) t -> p ko t", p=128)
            )
            nc.sync.dma_start(
                out=wv_sb, in_=wv[:].rearrange("(ko p) n -> p ko n", p=128)
            )
            nc.sync.dma_start(
                out=wp_sb, in_=wp[:].rearrange("(ko p) n -> p ko n", p=128)
            )
            # bias broadcast to all 128 partitions, f32, built once
            ones1_sb = wpool.tile([1, 128], bf16, name="ones1_sb")
            nc.gpsimd.memset(ones1_sb, 1.0)
            bias_bc = wpool.tile([128, C], f32, name="bias_bc")
            for half in range(2):
                psb = psB.tile([128, 512], f32, name="psb", tag="ps1")
                nc.tensor.matmul(
                    psb, ones1_sb, bp1_sb[:, ts(half, 512)], start=True, stop=True
                )
                nc.vector.tensor_copy(out=bias_bc[:, ts(half, 512)], in_=psb)
            # K^T pair-packed: [p, m, s] holds head 2m rows d=p on partitions
            # 0:64 and head 2m+1 rows d=p-64 on 64:128. Two persistent slots
            # for cross-batch overlap.
            kT2_tiles = [
                wpool.tile([128, MO, T], bf16, name=f"kT2_{slot}") for slot in range(2)
            ]

            def emit_q_chunk(m, xT8s, qTs):
                ps = psB.tile([128, T], f32, name="ps_q", tag="ps1")
                for k2 in range(KO2):
                    nc.tensor.matmul(
                        ps,
                        wq8_sb[:, 2 * k2 : 2 * k2 + 2, ts(m, 128)],
                        xT8s[:, 2 * k2 : 2 * k2 + 2, :],
                        start=(k2 == 0),
                        stop=(k2 == KO2 - 1),
                        perf_mode=DR,
                    )
                nc.scalar.copy(out=qTs[:, m, :], in_=ps)

            def emit_k_chunk(m, xT8s, kT2s):
                ps = psB.tile([128, T], f32, name="ps_k", tag="ps1")
                for k2 in range(KO2):
                    nc.tensor.matmul(
                        ps,
                        wk8_sb[:, 2 * k2 : 2 * k2 + 2, ts(m, 128)],
                        xT8s[:, 2 * k2 : 2 * k2 + 2, :],
                        start=(k2 == 0),
                        stop=(k2 == KO2 - 1),
                        perf_mode=DR,
                    )
                nc.vector.tensor_copy(out=kT2s[:, m, :], in_=ps)

            def emit_v_chunk(c, xTs, vs):
                # V: [s, h, 65] with ones column at d=64; chunk c = (i, half)
                i, half = c // 2, c % 2
                ps = psB.tile([128, 512], f32, name="ps_v", tag="ps1")
                for k in range(KO):
                    nc.tensor.matmul(
                        ps,
                        xTs[:, k, ts(i, 128)],
                        wv_sb[:, k, ts(half, 512)],
                        start=(k == 0),
                        stop=(k == KO - 1),
                    )
                nc.scalar.copy(
                    out=vs[:, i, 8 * half : 8 * half + 8, 0:64],
                    in_=ps.rearrange("p (h d) -> p h d", d=64),
                )

            def load_batch_inputs(bb):
                xT8s = xpool.tile([128, KO, T], f8, name="xT8_sb", tag="xT8")
                xTs = xpool.tile([128, KO, T], bf16, name="xT_sb", tag="xT")
                nc.sync.dma_start(
                    out=xT8s, in_=xT8[bb].rearrange("(ko p) t -> p ko t", p=128)
                )
                nc.sync.dma_start(
                    out=xTs, in_=xT[bb].rearrange("(ko p) t -> p ko t", p=128)
                )
                return xT8s, xTs

            def emit_proj_chunk(m, outTs, bb, out_sb_holder):
                # final projection chunk (i = m//2, half = m%2) of batch bb;
                # bias added during PSUM evacuation, DMA out after each i.
                i, half = m // 2, m % 2
                if half == 0:
                    out_sb_holder[0] = opool.tile(
                        [128, C], bf16, name="out_sb", tag="out_sb"
                    )
                out_sb = out_sb_holder[0]
                psF = psB.tile([128, 512], f32, name="psF", tag="ps1")
                for k in range(MO):
                    nc.tensor.matmul(
                        psF,
                        outTs[:, k, ts(i, 128)],
                        wp_sb[:, k, ts(half, 512)],
                        start=(k == 0),
                        stop=(k == MO - 1),
                    )
                nc.vector.tensor_add(
                    out=out_sb[:, ts(half, 512)],
                    in0=psF,
                    in1=bias_bc[:, ts(half, 512)],
                )
                if half == 1:
                    nc.sync.dma_start(out=out[bb, ts(i, 128), :], in_=out_sb)

            # ---- prologue: batch 0's full QKV (dense block) ----
            qT_sb = xpool.tile([128, MO, T], bf16, name="qT_sb", tag="qT")
            for m in range(MO):
                emit_q_chunk(m, xT80_sb, qT_sb)
            for m in range(MO):
                emit_k_chunk(m, xT80_sb, kT2_tiles[0])
            v_sb = xpool.tile([128, TCH, H, 65], bf16, name="v_sb", tag="v")
            nc.vector.memset(v_sb[:, :, :, 64:65], 1.0)
            # only heads 0-7's V chunks up front; heads 8-15's are interleaved
            # into batch 0's early attention pairs as PE filler (batch 0 has
            # no previous-batch projection to interleave)
            for c in (0, 2, 4, 6):
                emit_v_chunk(c, xT0_sb, v_sb)

            prev = None  # (outT tile, batch idx) pending final projection
            for b in range(b_loc):
                kT2 = kT2_tiles[b % 2]
                nb = b + 1
                if nb < b_loc:
                    xT8_nb, xT_nb = load_batch_inputs(nb)
                    qT_nb = xpool.tile([128, MO, T], bf16, name="qT_sb", tag="qT")
                    v_nb = xpool.tile([128, TCH, H, 65], bf16, name="v_sb", tag="v")
                    nc.vector.memset(v_nb[:, :, :, 64:65], 1.0)

                outT_sb = opool.tile([128, MO, T], bf16, name="outT_sb", tag="outT")
                on4 = opool.tile([128, TCH, HD], bf16, name="on4", tag="on4")
                out_sb_holder = [None]
                for m in range(MO):
                    # scores^T for heads (2m, 2m+1) as K=64 row-tiled pairs:
                    # head 2m streams through PE rows 0:63, head 2m+1 through
                    # rows 64:127, concurrently.
                    psS = [
                        psA.tile([128, PACK], f32, name=f"psS{half}", tag="psS")
                        for half in range(2)
                    ]
                    for j in range(TCH):
                        for half in range(2):
                            nc.tensor.matmul(
                                psS[half][:, ds(128 * j, 128)],
                                kT2[ds(64 * half, 64), m, ds(128 * j, 128)],
                                qT_sb[ds(64 * half, 64), m, ds(128 * j, 128)],
                                start=True,
                                stop=True,
                                tile_position=(64 * half, 0),
                            )
                        if j < TCH - 1:
                            for half in range(2):
                                nc.tensor.matmul(
                                    psS[half][:, ds(OD[j], ODW[j])],
                                    kT2[ds(64 * half, 64), m, ds(128 * j, 128)],
                                    qT_sb[ds(64 * half, 64), m, ds(128 * (j + 1), ODW[j])],
                                    start=True,
                                    stop=True,
                                    tile_position=(64 * half, 0),
                                )
                    # independent previous-batch projection filler: runs on
                    # the PE while this pair's exp/mask chain is still on
                    # scalar/vector
                    if prev is not None:
                        emit_proj_chunk(m, prev[0], prev[1], out_sb_holder)
                    elif m < 4:
                        emit_v_chunk((1, 3, 5, 7)[m], xT0_sb, v_sb)
                    elif nb < b_loc:
                        emit_q_chunk(m - 4, xT8_nb, qT_nb)
                    for half in range(2):
                        h = 2 * m + half
                        aT = apool.tile([128, PACK], bf16, name="aT", tag="aT")
                        nc.scalar.activation(aT, psS[half], AF.Exp, scale=EXP_SCALE)
                        # zero the masked (s>t) part of all 4 diagonal blocks
                        # in one multiply (they're packed contiguously); DVE
                        # (~350ns) over gpsimd (~790ns) -- this sits on the
                        # serial exp->mask->AV chain gating the PE
                        nc.vector.tensor_mul(
                            aT[:, 0:512], aT[:, 0:512], mask4_sb
                        )
                        # attn @ [V | 1]: all 4 t-chunks accumulate into ONE
                        # PSUM bank; col 64 of each chunk = softmax row-sum
                        psAV = psB.tile([128, TCH, 65], f32, name="psAV", tag="ps1")
                        for i in range(TCH):
                            for j in range(i + 1):
                                nc.tensor.matmul(
                                    psAV[:, i, :],
                                    aT[:, ds(av_block(i, j), 128)],
                                    v_sb[:, j, h, :],
                                    start=(i == 0 and j == 0),
                                    stop=(i == TCH - 1 and j == i),
                                )
                        rr = spool.tile([128, TCH], f32, name="rr", tag="rr")
                        nc.vector.reciprocal(rr, psAV[:, :, 64])
                        nc.vector.tensor_mul(
                            on4[:, :, ds(64 * h, 64)],
                            psAV[:, :, 0:64],
                            rr.broadcast_to([128, TCH, 64]),
                        )
                # batched head-concat transpose: [t, hd] -> [hd, t] per
                # t-chunk. On the SYNC queue: on the scalar queue these 1.3us
                # triggers block the Q/V evacuation copies behind them, which
                # hold PSUM slots and stall the PE mid-QKV-block.
                # The last batch's transposes are interleaved with its
                # projection in the tail instead.
                if nb < b_loc:
                    for i in range(TCH):
                        nc.sync.dma_start_transpose(
                            out=outT_sb[:, :, ts(i, 128)], in_=on4[:, i, :]
                        )

                # ---- rest of next batch's QKV as a dense PE block ----
                if nb < b_loc:
                    for m in range(4 if b == 0 else 0, MO):
                        emit_q_chunk(m, xT8_nb, qT_nb)
                    for m in range(MO):
                        emit_k_chunk(m, xT8_nb, kT2_tiles[nb % 2])
                    for c in range(8):
                        emit_v_chunk(c, xT_nb, v_nb)
                    qT_sb, v_sb = qT_nb, v_nb

                prev = (outT_sb, b)
                last_on4 = on4

            # ---- tail: last batch's final projection, transposes
            # interleaved so proj chunk i only waits for transpose i ----
            out_sb_holder = [None]
            for m in range(MO):
                if m % 2 == 0:
                    i = m // 2
                    # scalar queue: free in the tail (exps done) while the
                    # sync queue still drains output DMA triggers
                    nc.scalar.dma_start_transpose(
                        out=prev[0][:, :, ts(i, 128)], in_=last_on4[:, i, :]
                    )
                emit_proj_chunk(m, prev[0], prev[1], out_sb_holder)

    nc.compile()
    return nc


def make_in_maps(x, wq, wk, wv, w_proj, b_proj, b_loc=B_LOC, ncores=NCORES):
    bf16 = ml_dtypes.bfloat16
    f8 = ml_dtypes.float8_e4m3
    x = np.asarray(x, dtype=np.float32)
    # host-side layout prep (transpose / reshape / cast only)
    xTf = np.ascontiguousarray(x.transpose(0, 2, 1))  # [B, C, T] f32
    xT = xTf.astype(bf16)
    xT8 = xTf.astype(f8)
    wq2 = np.ascontiguousarray(
        np.asarray(wq, np.float32).transpose(1, 0, 2).reshape(C, HD)
    )
    wk2 = np.ascontiguousarray(
        np.asarray(wk, np.float32).transpose(1, 0, 2).reshape(C, HD)
    )
    wq8 = (wq2 * SW).astype(f8)
    wk8 = (wk2 * SW).astype(f8)
    wv2 = np.ascontiguousarray(
        np.asarray(wv, np.float32).transpose(1, 0, 2).reshape(C, HD)
    ).astype(bf16)
    wp2 = np.ascontiguousarray(np.asarray(w_proj, np.float32)).astype(bf16)
    bp2 = np.asarray(b_proj, np.float32).reshape(1, C).astype(bf16)
    # mask[p, f] = 1 where p <= f%128 (valid: s_in <= t_in on diagonal
    # blocks), tiled 4x horizontally for the packed diagonal region
    m1 = np.triu(np.ones((128, 128), np.float32))
    mask4 = np.tile(m1, (1, 4)).astype(bf16)
    in_maps = []
    for c in range(ncores):
        in_maps.append(
            {
                "xT": xT[c * b_loc : (c + 1) * b_loc],
                "xT8": xT8[c * b_loc : (c + 1) * b_loc],
                "wq8": wq8,
                "wk8": wk8,
                "wv": wv2,
                "wp": wp2,
                "bp": bp2,
                "mask4": mask4,
            }
        )
    return in_maps


def kernel(x, wq, wk, wv, w_proj, b_proj, **run_kwargs):
    from concourse import bass_utils

    if "nc" not in _CACHE:
        _CACHE["nc"] = build_nc(B_LOC)
    nc = _CACHE["nc"]
    in_maps = make_in_maps(x, wq, wk, wv, w_proj, b_proj)
    res = bass_utils.run_bass_kernel_spmd(
        nc, in_maps, core_ids=list(range(NCORES)), **run_kwargs
    )
    outs = [r["out"] for r in res.results]
    full = np.concatenate(outs, axis=0).astype(np.float32)
    if run_kwargs:
        _CACHE["last_result"] = res
    return full


# revision 31
# speedup vs baseline: 1.1589x; 1.1589x over previous
"""Causal multi-head attention (B=32,T=512,C=1024,H=16,D=64) on 8 TRN2 cores.

Strategy: pure data-parallel over the batch axis (4 batches per core, no
collectives). Per core, per batch:
  - Q^T/K^T projections run in fp8(e4m3) DoubleRow mode: contraction 256 per
    pass (2x fewer PE matmuls). fp8 noise only perturbs attention logits
    (sigma~0.25) so end-to-end rel-err stays ~1.3e-2 (gate 2e-2). Weights are
    pre-scaled x32 on the host; the exp() activation scale divides it back out.
  - V and the output projection stay bf16 (their noise hits the output
    directly).
  - K^T is stored pair-packed: head 2m on partitions 0:64, head 2m+1 on
    64:128. scores^T then runs as K=64 row-tiled matmul PAIRS
    (tile_position (0,0)/(64,0)) -- two heads stream concurrently through
    disjoint row-groups of the PE array, ~2x scores throughput, and no
    zero-padding matmul waste.
  - scores^T packs the 4 causal diagonal blocks contiguously at [0,512) so
    the 0/1 triangular mask is ONE gpsimd multiply per head (not 4).
  - softmax without max-subtraction (logits bounded); attn@[V|1] accumulates
    all 4 t-chunks of a head into a single PSUM bank, so normalization is one
    batched reciprocal + one broadcast tensor_mul per head (stride-0 AP).
  - head-concat transpose via one batched DMA-transpose per t-chunk; final
    projection with bias folded in via a K=128 matmul; fp32 output.
"""

import sys

if "/opt/trn_rl_repo" not in sys.path:
    sys.path.insert(0, "/opt/trn_rl_repo")

import numpy as np
import ml_dtypes

B, T, C = 32, 512, 1024
H, D = 16, 64
HD = H * D
NCORES = 8
B_LOC = B // NCORES
SW = 32.0  # host-side prescale of wq/wk before fp8 cast

_CACHE = {}


def build_nc(b_loc=B_LOC):
    import concourse.mybir as mybir
    from concourse import bacc
    from concourse.bass import ds, ts
    from concourse.tile import TileContext

    f32 = mybir.dt.float32
    bf16 = mybir.dt.bfloat16
    f8 = mybir.dt.float8e4
    AF = mybir.ActivationFunctionType
    DR = mybir.MatmulPerfMode.DoubleRow

    KO = C // 128  # 8 contraction chunks
    KO2 = KO // 2  # 4 DoubleRow chunks (K=256 each)
    MO = HD // 128  # 8 output-row chunks
    TCH = T // 128  # 4 t-chunks
    EXP_SCALE = 1.0 / (float(np.sqrt(C)) * SW * SW)

    # scores^T causal packing, diagonal-blocks-first:
    #   cols [128j, 128j+128)    : diagonal block of s-chunk j  (j=0..3)
    #   cols [OD[j], OD[j]+ODW[j]): off-diagonal strip of s-chunk j covering
    #                              t in [128(j+1), T)           (j=0..2)
    # Bank layout (2KB fp32 = 512 cols): [0,512) bank0; [512,896)+[896,1024)
    # bank1; [1024,1280) bank2 -- no matmul output crosses a bank boundary.
    OD = [512, 1024, 896]
    ODW = [384, 256, 128]
    PACK = 1280

    def av_block(i, j):
        # column offset of the aT block for (t-chunk i, s-chunk j), j<=i
        return 128 * i if i == j else OD[j] + 128 * (i - j - 1)

    nc = bacc.Bacc("TRN2", target_bir_lowering=False)
    xT = nc.dram_tensor("xT", [b_loc, C, T], bf16, kind="ExternalInput")
    xT8 = nc.dram_tensor("xT8", [b_loc, C, T], f8, kind="ExternalInput")
    wq8 = nc.dram_tensor("wq8", [C, HD], f8, kind="ExternalInput")
    wk8 = nc.dram_tensor("wk8", [C, HD], f8, kind="ExternalInput")
    wv = nc.dram_tensor("wv", [C, HD], bf16, kind="ExternalInput")
    wp = nc.dram_tensor("wp", [C, C], bf16, kind="ExternalInput")
    bp = nc.dram_tensor("bp", [1, C], bf16, kind="ExternalInput")
    mask4 = nc.dram_tensor("mask4", [128, 512], bf16, kind="ExternalInput")
    out = nc.dram_tensor("out", [b_loc, T, C], bf16, kind="ExternalOutput")

    with TileContext(nc) as tc:
        with (
            tc.tile_pool(name="weights", bufs=1) as wpool,
            tc.tile_pool(name="acts", bufs=2) as xpool,
            tc.tile_pool(name="attn", bufs=4) as apool,
            tc.tile_pool(name="small", bufs=8) as spool,
            tc.tile_pool(name="outs", bufs=2) as opool,
            tc.tile_pool(name="psS", bufs=2, space="PSUM") as psA,
            tc.tile_pool(name="ps1", bufs=2, space="PSUM") as psB,
        ):
            # ---- persistent weights ----
            # DMA order matters for the pipeline head: tiny tensors first,
            # then batch 0's fp8 x^T interleaved with wq8 so the first Q
            # matmul chain starts as soon as its operands land.
            wq8_sb = wpool.tile([128, KO, HD], f8, name="wq8_sb")
            wk8_sb = wpool.tile([128, KO, HD], f8, name="wk8_sb")
            wv_sb = wpool.tile([128, KO, HD], bf16, name="wv_sb")
            wp_sb = wpool.tile([128, KO, C], bf16, name="wp_sb")
            xT80_sb = xpool.tile([128, KO, T], f8, name="xT80_sb", tag="xT8")
            xT0_sb = xpool.tile([128, KO, T], bf16, name="xT0_sb", tag="xT")
            bp1_sb = wpool.tile([1, C], bf16, name="bp1_sb")
            nc.sync.dma_start(out=bp1_sb, in_=bp[:])
            mask4_sb = wpool.tile([128, 512], bf16, name="mask4_sb")
            nc.sync.dma_start(out=mask4_sb, in_=mask4[:])
            # per-chunk DMAs spread across 16 DMA engines (one big DMA would
            # serialize on a single queue); triggers split across the sync
            # and gpsimd queues since each trigger costs ~0.5us of queue time
            for k in range(KO):
                nc.sync.dma_start(out=xT80_sb[:, k, :], in_=xT8[0, ds(128 * k, 128), :])
                nc.gpsimd.dma_start(out=wq8_sb[:, k, :], in_=wq8[ds(128 * k, 128), :])
            for k in range(KO):
                nc.gpsimd.dma_start(out=wk8_sb[:, k, :], in_=wk8[ds(128 * k, 128), :])
            for k in range(KO):
                nc.sync.dma_start(out=xT0_sb[:, k, :], in_=xT[0, ds(128 * k, 128), :])
                nc.gpsimd.dma_start(out=wv_sb[:, k, :], in_=wv[ds(128 * k, 128), :])
            nc.sync.dma_start(
                out=wp_sb, in_=wp[:].rearrange("(ko p) n -> p ko n", p=128)
            )
            # bias broadcast to all 128 partitions, f32, built once
            ones1_sb = wpool.tile([1, 128], bf16, name="ones1_sb")
            nc.gpsimd.memset(ones1_sb, 1.0)
            bias_bc = wpool.tile([128, C], f32, name="bias_bc")
            for half in range(2):
                psb = psB.tile([128, 512], f32, name="psb", tag="ps1")
                nc.tensor.matmul(
                    psb, ones1_sb, bp1_sb[:, ts(half, 512)], start=True, stop=True
                )
                nc.vector.tensor_copy(out=bias_bc[:, ts(half, 512)], in_=psb)
            # ~4us of throwaway matmuls during the DMA head: sustained PE
            # activity flips the HAM clock-gate to full rate before the real
            # projection chains start
            for w in range(8):
                psw = psB.tile([128, 512], f32, name="psw", tag="ps1")
                nc.tensor.matmul(
                    psw, ones1_sb, bp1_sb[:, 0:512], start=True, stop=True
                )
            # K^T pair-packed: [p, m, s] holds head 2m rows d=p on partitions
            # 0:64 and head 2m+1 rows d=p-64 on 64:128. Two persistent slots
            # for cross-batch overlap.
            kT2_tiles = [
                wpool.tile([128, MO, T], bf16, name=f"kT2_{slot}") for slot in range(2)
            ]

            def emit_q_chunk(m, xT8s, qTs):
                ps = psB.tile([128, T], f32, name="ps_q", tag="ps1")
                for k2 in range(KO2):
                    nc.tensor.matmul(
                        ps,
                        wq8_sb[:, 2 * k2 : 2 * k2 + 2, ts(m, 128)],
                        xT8s[:, 2 * k2 : 2 * k2 + 2, :],
                        start=(k2 == 0),
                        stop=(k2 == KO2 - 1),
                        perf_mode=DR,
                    )
                nc.scalar.copy(out=qTs[:, m, :], in_=ps)

            def emit_k_chunk(m, xT8s, kT2s):
                ps = psB.tile([128, T], f32, name="ps_k", tag="ps1")
                for k2 in range(KO2):
                    nc.tensor.matmul(
                        ps,
                        wk8_sb[:, 2 * k2 : 2 * k2 + 2, ts(m, 128)],
                        xT8s[:, 2 * k2 : 2 * k2 + 2, :],
                        start=(k2 == 0),
                        stop=(k2 == KO2 - 1),
                        perf_mode=DR,
                    )
                nc.vector.tensor_copy(out=kT2s[:, m, :], in_=ps)

            def emit_v_chunk(c, xTs, vs):
                # V: [s, h, 65] with ones column at d=64; chunk c = (i, half)
                i, half = c // 2, c % 2
                ps = psB.tile([128, 512], f32, name="ps_v", tag="ps1")
                for k in range(KO):
                    nc.tensor.matmul(
                        ps,
                        xTs[:, k, ts(i, 128)],
                        wv_sb[:, k, ts(half, 512)],
                        start=(k == 0),
                        stop=(k == KO - 1),
                    )
                nc.scalar.copy(
                    out=vs[:, i, 8 * half : 8 * half + 8, 0:64],
                    in_=ps.rearrange("p (h d) -> p h d", d=64),
                )

            def load_batch_inputs(bb):
                xT8s = xpool.tile([128, KO, T], f8, name="xT8_sb", tag="xT8")
                xTs = xpool.tile([128, KO, T], bf16, name="xT_sb", tag="xT")
                for k in range(KO):
                    nc.gpsimd.dma_start(
                        out=xT8s[:, k, :], in_=xT8[bb, ds(128 * k, 128), :]
                    )
                for k in range(KO):
                    nc.gpsimd.dma_start(
                        out=xTs[:, k, :], in_=xT[bb, ds(128 * k, 128), :]
                    )
                return xT8s, xTs

            def emit_proj_chunk(m, outTs, bb, out_sb_holder):
                # final projection chunk (i = m//2, half = m%2) of batch bb;
                # bias added during PSUM evacuation, DMA out after each i.
                i, half = m // 2, m % 2
                if half == 0:
                    out_sb_holder[0] = opool.tile(
                        [128, C], bf16, name="out_sb", tag="out_sb"
                    )
                out_sb = out_sb_holder[0]
                psF = psB.tile([128, 512], f32, name="psF", tag="ps1")
                for k in range(MO):
                    nc.tensor.matmul(
                        psF,
                        outTs[:, k, ts(i, 128)],
                        wp_sb[:, k, ts(half, 512)],
                        start=(k == 0),
                        stop=(k == MO - 1),
                    )
                nc.vector.tensor_add(
                    out=out_sb[:, ts(half, 512)],
                    in0=psF,
                    in1=bias_bc[:, ts(half, 512)],
                )
                if half == 1:
                    nc.sync.dma_start(out=out[bb, ts(i, 128), :], in_=out_sb)

            # ---- prologue: batch 0's full QKV (dense block) ----
            qT_sb = xpool.tile([128, MO, T], bf16, name="qT_sb", tag="qT")
            for m in range(MO):
                emit_q_chunk(m, xT80_sb, qT_sb)
            for m in range(MO):
                emit_k_chunk(m, xT80_sb, kT2_tiles[0])
            v_sb = xpool.tile([128, TCH, H, 65], bf16, name="v_sb", tag="v")
            nc.vector.memset(v_sb[:, :, :, 64:65], 1.0)
            # only heads 0-7's V chunks up front; heads 8-15's are interleaved
            # into batch 0's early attention pairs as PE filler (batch 0 has
            # no previous-batch projection to interleave)
            for c in (0, 2, 4, 6):
                emit_v_chunk(c, xT0_sb, v_sb)

            prev = None  # (outT tile, batch idx) pending final projection
            for b in range(b_loc):
                kT2 = kT2_tiles[b % 2]
                nb = b + 1
                if nb < b_loc:
                    xT8_nb, xT_nb = load_batch_inputs(nb)
                    qT_nb = xpool.tile([128, MO, T], bf16, name="qT_sb", tag="qT")
                    v_nb = xpool.tile([128, TCH, H, 65], bf16, name="v_sb", tag="v")
                    nc.vector.memset(v_nb[:, :, :, 64:65], 1.0)

                outT_sb = opool.tile([128, MO, T], bf16, name="outT_sb", tag="outT")
                on4 = opool.tile([128, TCH, HD], bf16, name="on4", tag="on4")
                out_sb_holder = [None]
                for m in range(MO):
                    # scores^T for heads (2m, 2m+1) as K=64 row-tiled pairs:
                    # head 2m streams through PE rows 0:63, head 2m+1 through
                    # rows 64:127, concurrently.
                    psS = [
                        psA.tile([128, PACK], f32, name=f"psS{half}", tag="psS")
                        for half in range(2)
                    ]
                    for j in range(TCH):
                        for half in range(2):
                            nc.tensor.matmul(
                                psS[half][:, ds(128 * j, 128)],
                                kT2[ds(64 * half, 64), m, ds(128 * j, 128)],
                                qT_sb[ds(64 * half, 64), m, ds(128 * j, 128)],
                                start=True,
                                stop=True,
                                tile_position=(64 * half, 0),
                            )
                        if j < TCH - 1:
                            for half in range(2):
                                nc.tensor.matmul(
                                    psS[half][:, ds(OD[j], ODW[j])],
                                    kT2[ds(64 * half, 64), m, ds(128 * j, 128)],
                                    qT_sb[ds(64 * half, 64), m, ds(128 * (j + 1), ODW[j])],
                                    start=True,
                                    stop=True,
                                    tile_position=(64 * half, 0),
                                )
                    # independent previous-batch projection filler: runs on
                    # the PE while this pair's exp/mask chain is still on
                    # scalar/vector
                    if prev is not None:
                        emit_proj_chunk(m, prev[0], prev[1], out_sb_holder)
                    elif m < 4:
                        emit_v_chunk((1, 3, 5, 7)[m], xT0_sb, v_sb)
                    elif nb < b_loc:
                        emit_q_chunk(m - 4, xT8_nb, qT_nb)
                    for half in range(2):
                        h = 2 * m + half
                        aT = apool.tile([128, PACK], bf16, name="aT", tag="aT")
                        nc.scalar.activation(aT, psS[half], AF.Exp, scale=EXP_SCALE)
                        # zero the masked (s>t) part of all 4 diagonal blocks
                        # in one multiply (they're packed contiguously); DVE
                        # (~350ns) over gpsimd (~790ns) -- this sits on the
                        # serial exp->mask->AV chain gating the PE
                        nc.vector.tensor_mul(
                            aT[:, 0:512], aT[:, 0:512], mask4_sb
                        )
                        # attn @ [V | 1]: all 4 t-chunks accumulate into ONE
                        # PSUM bank; col 64 of each chunk = softmax row-sum
                        psAV = psB.tile([128, TCH, 65], f32, name="psAV", tag="ps1")
                        for i in range(TCH):
                            for j in range(i + 1):
                                nc.tensor.matmul(
                                    psAV[:, i, :],
                                    aT[:, ds(av_block(i, j), 128)],
                                    v_sb[:, j, h, :],
                                    start=(i == 0 and j == 0),
                                    stop=(i == TCH - 1 and j == i),
                                )
                        rr = spool.tile([128, TCH], f32, name="rr", tag="rr")
                        nc.vector.reciprocal(rr, psAV[:, :, 64])
                        nc.vector.tensor_mul(
                            on4[:, :, ds(64 * h, 64)],
                            psAV[:, :, 0:64],
                            rr.broadcast_to([128, TCH, 64]),
                        )
                # batched head-concat transpose: [t, hd] -> [hd, t] per
                # t-chunk. On the SYNC queue: on the scalar queue these 1.3us
                # triggers block the Q/V evacuation copies behind them, which
                # hold PSUM slots and stall the PE mid-QKV-block.
                # The last batch's transposes are interleaved with its
                # projection in the tail instead.
                if nb < b_loc:
                    for i in range(TCH):
                        nc.sync.dma_start_transpose(
                            out=outT_sb[:, :, ts(i, 128)], in_=on4[:, i, :]
                        )

                # ---- rest of next batch's QKV as a dense PE block ----
                if nb < b_loc:
                    for m in range(4 if b == 0 else 0, MO):
                        emit_q_chunk(m, xT8_nb, qT_nb)
                    for m in range(MO):
                        emit_k_chunk(m, xT8_nb, kT2_tiles[nb % 2])
                    for c in range(8):
                        emit_v_chunk(c, xT_nb, v_nb)
                    qT_sb, v_sb = qT_nb, v_nb

                prev = (outT_sb, b)
                last_on4 = on4

            # ---- tail: last batch's final projection, transposes
            # interleaved so proj chunk i only waits for transpose i ----
            out_sb_holder = [None]
            for m in range(MO):
                if m % 2 == 0:
                    i = m // 2
                    # scalar queue: free in the tail (exps done) while the
                    # sync queue still drains output DMA triggers
                    nc.scalar.dma_start_transpose(
                        out=prev[0][:, :, ts(i, 128)], in_=last_on4[:, i, :]
                    )
                emit_proj_chunk(m, prev[0], prev[1], out_sb_holder)

    nc.compile()
    return nc


def make_in_maps(x, wq, wk, wv, w_proj, b_proj, b_loc=B_LOC, ncores=NCORES):
    bf16 = ml_dtypes.bfloat16
    f8 = ml_dtypes.float8_e4m3
    x = np.asarray(x, dtype=np.float32)
    # host-side layout prep (transpose / reshape / cast only)
    xTf = np.ascontiguousarray(x.transpose(0, 2, 1))  # [B, C, T] f32
    xT = xTf.astype(bf16)
    xT8 = xTf.astype(f8)
    wq2 = np.ascontiguousarray(
        np.asarray(wq, np.float32).transpose(1, 0, 2).reshape(C, HD)
    )
    wk2 = np.ascontiguousarray(
        np.asarray(wk, np.float32).transpose(1, 0, 2).reshape(C, HD)
    )
    wq8 = (wq2 * SW).astype(f8)
    wk8 = (wk2 * SW).astype(f8)
    wv2 = np.ascontiguousarray(
        np.asarray(wv, np.float32).transpose(1, 0, 2).reshape(C, HD)
    ).astype(bf16)
    wp2 = np.ascontiguousarray(np.asarray(w_proj, np.float32)).astype(bf16)
    bp2 = np.asarray(b_proj, np.float32).reshape(1, C).astype(bf16)
    # mask[p, f] = 1 where p <= f%128 (valid: s_in <= t_in on diagonal
    # blocks), tiled 4x horizontally for the packed diagonal region
    m1 = np.triu(np.ones((128, 128), np.float32))
    mask4 = np.tile(m1, (1, 4)).astype(bf16)
    in_maps = []
    for c in range(ncores):
        in_maps.append(
            {
                "xT": xT[c * b_loc : (c + 1) * b_loc],
                "xT8": xT8[c * b_loc : (c + 1) * b_loc],
                "wq8": wq8,
                "wk8": wk8,
                "wv": wv2,
                "wp": wp2,
                "bp": bp2,
                "mask4": mask4,
            }
        )
    return in_maps


def kernel(x, wq, wk, wv, w_proj, b_proj, **run_kwargs):
    from concourse import bass_utils

    if "nc" not in _CACHE:
        _CACHE["nc"] = build_nc(B_LOC)
    nc = _CACHE["nc"]
    in_maps = make_in_maps(x, wq, wk, wv, w_proj, b_proj)
    res = bass_utils.run_bass_kernel_spmd(
        nc, in_maps, core_ids=list(range(NCORES)), **run_kwargs
    )
    outs = [r["out"] for r in res.results]
    full = np.concatenate(outs, axis=0).astype(np.float32)
    if run_kwargs:
        _CACHE["last_result"] = res
    return full


# revision 33
# speedup vs baseline: 1.1990x; 1.0345x over previous
"""Causal multi-head attention (B=32,T=512,C=1024,H=16,D=64) on 8 TRN2 cores.

Strategy: pure data-parallel over the batch axis (4 batches per core, no
collectives). Per core, per batch:
  - Q^T/K^T projections run in fp8(e4m3) DoubleRow mode: contraction 256 per
    pass (2x fewer PE matmuls). fp8 noise only perturbs attention logits
    (sigma~0.25) so end-to-end rel-err stays ~1.3e-2 (gate 2e-2). Weights are
    pre-scaled x32 on the host; the exp() activation scale divides it back out.
  - V and the output projection stay bf16 (their noise hits the output
    directly).
  - K^T is stored pair-packed: head 2m on partitions 0:64, head 2m+1 on
    64:128. scores^T then runs as K=64 row-tiled matmul PAIRS
    (tile_position (0,0)/(64,0)) -- two heads stream concurrently through
    disjoint row-groups of the PE array, ~2x scores throughput, and no
    zero-padding matmul waste.
  - scores^T packs the 4 causal diagonal blocks contiguously at [0,512) so
    the 0/1 triangular mask is ONE gpsimd multiply per head (not 4).
  - softmax without max-subtraction (logits bounded); attn@[V|1] accumulates
    all 4 t-chunks of a head into a single PSUM bank, so normalization is one
    batched reciprocal + one broadcast tensor_mul per head (stride-0 AP).
  - head-concat transpose via one batched DMA-transpose per t-chunk; final
    projection with bias folded in via a K=128 matmul; fp32 output.
"""

import sys

if "/opt/trn_rl_repo" not in sys.path:
    sys.path.insert(0, "/opt/trn_rl_repo")

import numpy as np
import ml_dtypes

B, T, C = 32, 512, 1024
H, D = 16, 64
HD = H * D
NCORES = 8
B_LOC = B // NCORES
SW = 32.0  # host-side prescale of wq/wk before fp8 cast

_CACHE = {}


def build_nc(b_loc=B_LOC):
    import concourse.mybir as mybir
    from concourse import bacc
    from concourse.bass import ds, ts
    from concourse.tile import TileContext

    f32 = mybir.dt.float32
    bf16 = mybir.dt.bfloat16
    f8 = mybir.dt.float8e4
    AF = mybir.ActivationFunctionType
    DR = mybir.MatmulPerfMode.DoubleRow

    KO = C // 128  # 8 contraction chunks
    KO2 = KO // 2  # 4 DoubleRow chunks (K=256 each)
    MO = HD // 128  # 8 output-row chunks
    TCH = T // 128  # 4 t-chunks
    EXP_SCALE = 1.0 / (float(np.sqrt(C)) * SW * SW)

    # scores^T causal packing, diagonal-blocks-first:
    #   cols [128j, 128j+128)    : diagonal block of s-chunk j  (j=0..3)
    #   cols [OD[j], OD[j]+ODW[j]): off-diagonal strip of s-chunk j covering
    #                              t in [128(j+1), T)           (j=0..2)
    # Bank layout (2KB fp32 = 512 cols): [0,512) bank0; [512,896)+[896,1024)
    # bank1; [1024,1280) bank2 -- no matmul output crosses a bank boundary.
    OD = [512, 1024, 896]
    ODW = [384, 256, 128]
    PACK = 1280

    def av_block(i, j):
        # column offset of the aT block for (t-chunk i, s-chunk j), j<=i
        return 128 * i if i == j else OD[j] + 128 * (i - j - 1)

    nc = bacc.Bacc("TRN2", target_bir_lowering=False)
    xT = nc.dram_tensor("xT", [b_loc, C, T], bf16, kind="ExternalInput")
    xT8 = nc.dram_tensor("xT8", [b_loc, C, T], f8, kind="ExternalInput")
    wq8 = nc.dram_tensor("wq8", [C, HD], f8, kind="ExternalInput")
    wk8 = nc.dram_tensor("wk8", [C, HD], f8, kind="ExternalInput")
    wv = nc.dram_tensor("wv", [C, HD], bf16, kind="ExternalInput")
    wp = nc.dram_tensor("wp", [C, C], bf16, kind="ExternalInput")
    bp = nc.dram_tensor("bp", [1, C], bf16, kind="ExternalInput")
    mask4 = nc.dram_tensor("mask4", [128, 512], bf16, kind="ExternalInput")
    out = nc.dram_tensor("out", [b_loc, T, C], bf16, kind="ExternalOutput")

    with TileContext(nc) as tc:
        with (
            tc.tile_pool(name="weights", bufs=1) as wpool,
            tc.tile_pool(name="acts", bufs=2) as xpool,
            tc.tile_pool(name="attn", bufs=4) as apool,
            tc.tile_pool(name="small", bufs=8) as spool,
            tc.tile_pool(name="outs", bufs=2) as opool,
            tc.tile_pool(name="psS", bufs=2, space="PSUM") as psA,
            tc.tile_pool(name="ps1", bufs=2, space="PSUM") as psB,
        ):
            # ---- persistent weights ----
            # DMA order matters for the pipeline head: tiny tensors first,
            # then batch 0's fp8 x^T interleaved with wq8 so the first Q
            # matmul chain starts as soon as its operands land.
            wq8_sb = wpool.tile([128, KO, HD], f8, name="wq8_sb")
            wk8_sb = wpool.tile([128, KO, HD], f8, name="wk8_sb")
            wv_sb = wpool.tile([128, KO, HD], bf16, name="wv_sb")
            wp_sb = wpool.tile([128, KO, C], bf16, name="wp_sb")
            xT80_sb = xpool.tile([128, KO, T], f8, name="xT80_sb", tag="xT8")
            xT0_sb = xpool.tile([128, KO, T], bf16, name="xT0_sb", tag="xT")
            bp1_sb = wpool.tile([1, C], bf16, name="bp1_sb")
            nc.sync.dma_start(out=bp1_sb, in_=bp[:])
            mask4_sb = wpool.tile([128, 512], bf16, name="mask4_sb")
            nc.sync.dma_start(out=mask4_sb, in_=mask4[:])
            # per-chunk DMAs spread across 16 DMA engines (one big DMA would
            # serialize on a single queue; gpsimd-triggered DMA uses the slow
            # software DGE path -- keep all triggers on the sync HWDGE queue)
            for k in range(KO):
                nc.sync.dma_start(out=xT80_sb[:, k, :], in_=xT8[0, ds(128 * k, 128), :])
                nc.sync.dma_start(out=wq8_sb[:, k, :], in_=wq8[ds(128 * k, 128), :])
            for k in range(KO):
                nc.sync.dma_start(out=wk8_sb[:, k, :], in_=wk8[ds(128 * k, 128), :])
            for k in range(KO):
                nc.sync.dma_start(out=xT0_sb[:, k, :], in_=xT[0, ds(128 * k, 128), :])
                nc.sync.dma_start(out=wv_sb[:, k, :], in_=wv[ds(128 * k, 128), :])
            nc.sync.dma_start(
                out=wp_sb, in_=wp[:].rearrange("(ko p) n -> p ko n", p=128)
            )
            # bias broadcast to all 128 partitions, f32, built once
            ones1_sb = wpool.tile([1, 128], bf16, name="ones1_sb")
            nc.gpsimd.memset(ones1_sb, 1.0)
            bias_bc = wpool.tile([128, C], f32, name="bias_bc")
            for half in range(2):
                psb = psB.tile([128, 512], f32, name="psb", tag="ps1")
                nc.tensor.matmul(
                    psb, ones1_sb, bp1_sb[:, ts(half, 512)], start=True, stop=True
                )
                nc.vector.tensor_copy(out=bias_bc[:, ts(half, 512)], in_=psb)
            # ~4us of throwaway matmuls during the DMA head: sustained PE
            # activity flips the HAM clock-gate to full rate before the real
            # projection chains start
            for w in range(8):
                psw = psB.tile([128, 512], f32, name="psw", tag="ps1")
                nc.tensor.matmul(
                    psw, ones1_sb, bp1_sb[:, 0:512], start=True, stop=True
                )
            # K^T pair-packed: [p, m, s] holds head 2m rows d=p on partitions
            # 0:64 and head 2m+1 rows d=p-64 on 64:128. Two persistent slots
            # for cross-batch overlap.
            kT2_tiles = [
                wpool.tile([128, MO, T], bf16, name=f"kT2_{slot}") for slot in range(2)
            ]

            def emit_q_chunk(m, xT8s, qTs):
                ps = psB.tile([128, T], f32, name="ps_q", tag="ps1")
                for k2 in range(KO2):
                    nc.tensor.matmul(
                        ps,
                        wq8_sb[:, 2 * k2 : 2 * k2 + 2, ts(m, 128)],
                        xT8s[:, 2 * k2 : 2 * k2 + 2, :],
                        start=(k2 == 0),
                        stop=(k2 == KO2 - 1),
                        perf_mode=DR,
                    )
                nc.scalar.copy(out=qTs[:, m, :], in_=ps)

            def emit_k_chunk(m, xT8s, kT2s):
                ps = psB.tile([128, T], f32, name="ps_k", tag="ps1")
                for k2 in range(KO2):
                    nc.tensor.matmul(
                        ps,
                        wk8_sb[:, 2 * k2 : 2 * k2 + 2, ts(m, 128)],
                        xT8s[:, 2 * k2 : 2 * k2 + 2, :],
                        start=(k2 == 0),
                        stop=(k2 == KO2 - 1),
                        perf_mode=DR,
                    )
                nc.vector.tensor_copy(out=kT2s[:, m, :], in_=ps)

            def emit_v_chunk(c, xTs, vs):
                # V: [s, h, 65] with ones column at d=64; chunk c = (i, half)
                i, half = c // 2, c % 2
                ps = psB.tile([128, 512], f32, name="ps_v", tag="ps1")
                for k in range(KO):
                    nc.tensor.matmul(
                        ps,
                        xTs[:, k, ts(i, 128)],
                        wv_sb[:, k, ts(half, 512)],
                        start=(k == 0),
                        stop=(k == KO - 1),
                    )
                nc.scalar.copy(
                    out=vs[:, i, 8 * half : 8 * half + 8, 0:64],
                    in_=ps.rearrange("p (h d) -> p h d", d=64),
                )

            def load_batch_inputs(bb):
                xT8s = xpool.tile([128, KO, T], f8, name="xT8_sb", tag="xT8")
                xTs = xpool.tile([128, KO, T], bf16, name="xT_sb", tag="xT")
                for k in range(KO):
                    nc.sync.dma_start(
                        out=xT8s[:, k, :], in_=xT8[bb, ds(128 * k, 128), :]
                    )
                for k in range(KO):
                    nc.sync.dma_start(
                        out=xTs[:, k, :], in_=xT[bb, ds(128 * k, 128), :]
                    )
                return xT8s, xTs

            def emit_proj_chunk(m, outTs, bb, out_sb_holder):
                # final projection chunk (i = m//2, half = m%2) of batch bb;
                # bias added during PSUM evacuation, DMA out after each i.
                i, half = m // 2, m % 2
                if half == 0:
                    out_sb_holder[0] = opool.tile(
                        [128, C], bf16, name="out_sb", tag="out_sb"
                    )
                out_sb = out_sb_holder[0]
                psF = psB.tile([128, 512], f32, name="psF", tag="ps1")
                for k in range(MO):
                    nc.tensor.matmul(
                        psF,
                        outTs[:, k, ts(i, 128)],
                        wp_sb[:, k, ts(half, 512)],
                        start=(k == 0),
                        stop=(k == MO - 1),
                    )
                nc.vector.tensor_add(
                    out=out_sb[:, ts(half, 512)],
                    in0=psF,
                    in1=bias_bc[:, ts(half, 512)],
                )
                if half == 1:
                    nc.sync.dma_start(out=out[bb, ts(i, 128), :], in_=out_sb)

            # ---- prologue: batch 0's full QKV (dense block) ----
            qT_sb = xpool.tile([128, MO, T], bf16, name="qT_sb", tag="qT")
            for m in range(MO):
                emit_q_chunk(m, xT80_sb, qT_sb)
            for m in range(MO):
                emit_k_chunk(m, xT80_sb, kT2_tiles[0])
            v_sb = xpool.tile([128, TCH, H, 65], bf16, name="v_sb", tag="v")
            nc.vector.memset(v_sb[:, :, :, 64:65], 1.0)
            # only heads 0-7's V chunks up front; heads 8-15's are interleaved
            # into batch 0's early attention pairs as PE filler (batch 0 has
            # no previous-batch projection to interleave)
            for c in (0, 2, 4, 6):
                emit_v_chunk(c, xT0_sb, v_sb)

            prev = None  # (outT tile, batch idx) pending final projection
            for b in range(b_loc):
                kT2 = kT2_tiles[b % 2]
                nb = b + 1
                if nb < b_loc:
                    xT8_nb, xT_nb = load_batch_inputs(nb)
                    qT_nb = xpool.tile([128, MO, T], bf16, name="qT_sb", tag="qT")
                    v_nb = xpool.tile([128, TCH, H, 65], bf16, name="v_sb", tag="v")
                    nc.vector.memset(v_nb[:, :, :, 64:65], 1.0)

                outT_sb = opool.tile([128, MO, T], bf16, name="outT_sb", tag="outT")
                on4 = opool.tile([128, TCH, HD], bf16, name="on4", tag="on4")
                out_sb_holder = [None]
                for m in range(MO):
                    # scores^T for heads (2m, 2m+1) as K=64 row-tiled pairs:
                    # head 2m streams through PE rows 0:63, head 2m+1 through
                    # rows 64:127, concurrently.
                    psS = [
                        psA.tile([128, PACK], f32, name=f"psS{half}", tag="psS")
                        for half in range(2)
                    ]
                    for j in range(TCH):
                        for half in range(2):
                            nc.tensor.matmul(
                                psS[half][:, ds(128 * j, 128)],
                                kT2[ds(64 * half, 64), m, ds(128 * j, 128)],
                                qT_sb[ds(64 * half, 64), m, ds(128 * j, 128)],
                                start=True,
                                stop=True,
                                tile_position=(64 * half, 0),
                            )
                        if j < TCH - 1:
                            for half in range(2):
                                nc.tensor.matmul(
                                    psS[half][:, ds(OD[j], ODW[j])],
                                    kT2[ds(64 * half, 64), m, ds(128 * j, 128)],
                                    qT_sb[ds(64 * half, 64), m, ds(128 * (j + 1), ODW[j])],
                                    start=True,
                                    stop=True,
                                    tile_position=(64 * half, 0),
                                )
                    # independent previous-batch projection filler: runs on
                    # the PE while this pair's exp/mask chain is still on
                    # scalar/vector
                    if prev is not None:
                        emit_proj_chunk(m, prev[0], prev[1], out_sb_holder)
                    elif m < 4:
                        emit_v_chunk((1, 3, 5, 7)[m], xT0_sb, v_sb)
                    elif nb < b_loc:
                        emit_q_chunk(m - 4, xT8_nb, qT_nb)
                    for half in range(2):
                        h = 2 * m + half
                        aT = apool.tile([128, PACK], bf16, name="aT", tag="aT")
                        nc.scalar.activation(aT, psS[half], AF.Exp, scale=EXP_SCALE)
                        # zero the masked (s>t) part of all 4 diagonal blocks
                        # in one multiply (they're packed contiguously); DVE
                        # (~350ns) over gpsimd (~790ns) -- this sits on the
                        # serial exp->mask->AV chain gating the PE
                        nc.vector.tensor_mul(
                            aT[:, 0:512], aT[:, 0:512], mask4_sb
                        )
                        # attn @ [V | 1]: all 4 t-chunks accumulate into ONE
                        # PSUM bank; col 64 of each chunk = softmax row-sum
                        psAV = psB.tile([128, TCH, 65], f32, name="psAV", tag="ps1")
                        for i in range(TCH):
                            for j in range(i + 1):
                                nc.tensor.matmul(
                                    psAV[:, i, :],
                                    aT[:, ds(av_block(i, j), 128)],
                                    v_sb[:, j, h, :],
                                    start=(i == 0 and j == 0),
                                    stop=(i == TCH - 1 and j == i),
                                )
                        rr = spool.tile([128, TCH], f32, name="rr", tag="rr")
                        nc.vector.reciprocal(rr, psAV[:, :, 64])
                        nc.vector.tensor_mul(
                            on4[:, :, ds(64 * h, 64)],
                            psAV[:, :, 0:64],
                            rr.broadcast_to([128, TCH, 64]),
                        )
                # batched head-concat transpose: [t, hd] -> [hd, t] per
                # t-chunk. On the SYNC queue: on the scalar queue these 1.3us
                # triggers block the Q/V evacuation copies behind them, which
                # hold PSUM slots and stall the PE mid-QKV-block.
                # The last batch's transposes are interleaved with its
                # projection in the tail instead.
                if nb < b_loc:
                    for i in range(TCH):
                        nc.sync.dma_start_transpose(
                            out=outT_sb[:, :, ts(i, 128)], in_=on4[:, i, :]
                        )

                # ---- rest of next batch's QKV as a dense PE block ----
                if nb < b_loc:
                    for m in range(4 if b == 0 else 0, MO):
                        emit_q_chunk(m, xT8_nb, qT_nb)
                    for m in range(MO):
                        emit_k_chunk(m, xT8_nb, kT2_tiles[nb % 2])
                    for c in range(8):
                        emit_v_chunk(c, xT_nb, v_nb)
                    qT_sb, v_sb = qT_nb, v_nb

                prev = (outT_sb, b)
                last_on4 = on4

            # ---- tail: last batch's final projection, transposes
            # interleaved so proj chunk i only waits for transpose i ----
            out_sb_holder = [None]
            for m in range(MO):
                if m % 2 == 0:
                    i = m // 2
                    # scalar queue: free in the tail (exps done) while the
                    # sync queue still drains output DMA triggers
                    nc.scalar.dma_start_transpose(
                        out=prev[0][:, :, ts(i, 128)], in_=last_on4[:, i, :]
                    )
                emit_proj_chunk(m, prev[0], prev[1], out_sb_holder)

    nc.compile()
    return nc


def make_in_maps(x, wq, wk, wv, w_proj, b_proj, b_loc=B_LOC, ncores=NCORES):
    bf16 = ml_dtypes.bfloat16
    f8 = ml_dtypes.float8_e4m3
    x = np.asarray(x, dtype=np.float32)
    # host-side layout prep (transpose / reshape / cast only)
    xTf = np.ascontiguousarray(x.transpose(0, 2, 1))  # [B, C, T] f32
    xT = xTf.astype(bf16)
    xT8 = xTf.astype(f8)
    wq2 = np.ascontiguousarray(
        np.asarray(wq, np.float32).transpose(1, 0, 2).reshape(C, HD)
    )
    wk2 = np.ascontiguousarray(
        np.asarray(wk, np.float32).transpose(1, 0, 2).reshape(C, HD)
    )
    wq8 = (wq2 * SW).astype(f8)
    wk8 = (wk2 * SW).astype(f8)
    wv2 = np.ascontiguousarray(
        np.asarray(wv, np.float32).transpose(1, 0, 2).reshape(C, HD)
    ).astype(bf16)
    wp2 = np.ascontiguousarray(np.asarray(w_proj, np.float32)).astype(bf16)
    bp2 = np.asarray(b_proj, np.float32).reshape(1, C).astype(bf16)
    # mask[p, f] = 1 where p <= f%128 (valid: s_in <= t_in on diagonal
    # blocks), tiled 4x horizontally for the packed diagonal region
    m1 = np.triu(np.ones((128, 128), np.float32))
    mask4 = np.tile(m1, (1, 4)).astype(bf16)
    in_maps = []
    for c in range(ncores):
        in_maps.append(
            {
                "xT": xT[c * b_loc : (c + 1) * b_loc],
                "xT8": xT8[c * b_loc : (c + 1) * b_loc],
                "wq8": wq8,
                "wk8": wk8,
                "wv": wv2,
                "wp": wp2,
                "bp": bp2,
                "mask4": mask4,
            }
        )
    return in_maps


def kernel(x, wq, wk, wv, w_proj, b_proj, **run_kwargs):
    from concourse import bass_utils

    if "nc" not in _CACHE:
        _CACHE["nc"] = build_nc(B_LOC)
    nc = _CACHE["nc"]
    in_maps = make_in_maps(x, wq, wk, wv, w_proj, b_proj)
    res = bass_utils.run_bass_kernel_spmd(
        nc, in_maps, core_ids=list(range(NCORES)), **run_kwargs
    )
    outs = [r["out"] for r in res.results]
    full = np.concatenate(outs, axis=0).astype(np.float32)
    if run_kwargs:
        _CACHE["last_result"] = res
    return full
